# revision 6
# baseline (speedup 1.0000x reference)
"""Grouped-GEMM (MoE expert FFN) kernel for 8 Trainium2 NeuronCores.

Problem: out[e, m, n] = sum_k x[e, m, k] * w[e, n, k] for m < m_sizes[e],
         zero elsewhere.  E=8, MAX_M=2048, K=2048, N=8192, fp32.

Strategy
--------
* N-split sharding: every core computes ALL experts' valid row-tiles
  against its own (N/8)-wide column slice of each expert's weights.
  Per-core work is identical by construction (perfect load balance) and
  per-core weight traffic hits the 64 MB floor (each weight element is
  read exactly once fleet-wide).
* Only ceil(m_e/128) row-tiles per expert are computed (the program is
  specialized to the runtime m_sizes tuple and cached per tuple).
* fp32r matmuls: operands are pre-rounded on the host to the hardware's
  fp32r format (round-half-up to 11 mantissa bits), which streams at
  1 cycle/row on the PE -- 4x faster than fp32 with ~1e-4 output error.
* Host pre-transposes AND pre-swizzles x/w so every device DMA is one
  large contiguous line per partition (64 KB for weights, 16 KB for x).
* PSUM: one [128, 512] bank per accumulation group, 8-deep pipelining;
  DVE evicts PSUM -> SBUF; outputs stream back per row-tile.
"""
import math
import os
import sys
import types

import numpy as np

import concourse.bass as bass
import concourse.tile as tile
from concourse import bacc, mybir
from concourse.bass_utils import run_bass_kernel_spmd

P = 128          # partition dim / k-tile
N_CORES = 8
MM_N = 512       # moving free dim per matmul (one PSUM bank of fp32)
M_CHUNK = 2      # row-tiles per x DMA
CH = M_CHUNK * P

LAST_RESULT = None   # BassKernelResults of the most recent run (for tests)


def _install_profile_shim():
    """The agent image's antenv stub lacks axon_hooks; provide it so
    BASS_TRACE=1 profiling works instead of crashing."""
    if "antenv.axon_hooks" in sys.modules:
        return
    try:
        from trn_agent_boot.trn_boot import _ntff_profile_via_ctypes
        hook = _ntff_profile_via_ctypes("/opt/axon/libaxon_pjrt.so")
        mod = types.ModuleType("antenv.axon_hooks")
        mod.get_axon_ntff_profile_hook = lambda: hook
        sys.modules["antenv.axon_hooks"] = mod
        import antenv
        antenv.axon_hooks = mod
    except Exception:
        pass


def round_fp32r(a: np.ndarray) -> np.ndarray:
    """Hardware fp32r rounding: round-half-up mantissa to 11 bits."""
    u = np.ascontiguousarray(a).view(np.uint32)
    r = (u + np.uint32(1 << 11)) & ~np.uint32((1 << 12) - 1)
    return r.view(np.float32)


def to_fp16(a: np.ndarray) -> np.ndarray:
    return np.ascontiguousarray(a).astype(np.float16)


def build_nc(t_list, K, NC_N, psum_bufs=8, w_bufs=20, ws_bufs=6, x_bufs=3,
             xs_bufs=3, out_bufs=4):
    """Build the SPMD program for per-segment row-tile counts t_list.

    Inputs arrive as fp16 (halves HBM traffic vs fp32) and are upconverted
    on-chip to fp32r: HW fp32r matmuls have a measured 227 ns steady-state
    cadence vs 259 ns for fp16 matmuls, so compute in fp32r but DMA fp16.
    w converts on the Act engine, x on Pool; both are far off the PE
    critical path.
    """
    KK = K // P
    NSEG = len(t_list)
    NH = NC_N // MM_N
    n_chunks = sum((t + M_CHUNK - 1) // M_CHUNK for t in t_list)
    R = P * sum(t_list)

    nc = bacc.Bacc("TRN2", target_bir_lowering=False, debug=False,
                   num_devices=N_CORES)
    xsw = nc.dram_tensor("xsw", [n_chunks * P, KK * CH], mybir.dt.float16,
                         kind="ExternalInput").ap()
    # per (segment, k-tile): row ((seg*KK + kk)*P + p)
    wsw = nc.dram_tensor("wsw", [NSEG * KK * P, NC_N], mybir.dt.float16,
                         kind="ExternalInput").ap()
    out = nc.dram_tensor("out", [R, NC_N], mybir.dt.float32,
                         kind="ExternalOutput").ap()

    with tile.TileContext(nc) as tc:
        with tc.tile_pool(name="wp", bufs=w_bufs) as wp, \
             tc.tile_pool(name="ws", bufs=ws_bufs) as ws, \
             tc.tile_pool(name="xp", bufs=x_bufs) as xp, \
             tc.tile_pool(name="xs", bufs=xs_bufs) as xs, \
             tc.tile_pool(name="op", bufs=out_bufs) as op, \
             tc.tile_pool(name="pp", bufs=psum_bufs, space="PSUM") as pp, \
             tc.tile_pool(name="wu", bufs=1) as wu:
            # PE warmup: dummy fp32r matmuls spanning the initial DMA wait
            # keep the HAM activity monitor engaged so the PE clock is at
            # 2.4 GHz (not the cold 1.2) when the first real tiles land.
            # Rotate across 4 PSUM tiles — a single tile WAW-chains and
            # the sem overhead makes the warmup a net loss.
            wa_f32 = wu.tile([P, MM_N], mybir.dt.float32, tag="wa")
            nc.gpsimd.memset(wa_f32[:], 0.0)
            wa_r = wu.tile([P, MM_N], mybir.dt.float32r, tag="war")
            nc.vector.tensor_copy(wa_r[:], wa_f32[:])
            wpss = [pp.tile([P, MM_N], mybir.dt.float32, tag="ps",
                            name="wps") for _ in range(4)]
            for i in range(20):
                nc.tensor.matmul(wpss[i % 4][:], wa_r[:, :P], wa_r[:],
                                 start=True, stop=True)
            row0 = 0
            chunk = 0
            for seg, T in enumerate(t_list):
                w_ts = []
                for kk in range(KK):
                    w_s = ws.tile([P, NC_N], mybir.dt.float16, tag="wst")
                    nc.sync.dma_start(
                        out=w_s[:],
                        in_=wsw[(seg * KK + kk) * P:
                                (seg * KK + kk + 1) * P, :])
                    w_t = wp.tile([P, NC_N], mybir.dt.float32r, tag="w")
                    nc.scalar.copy(w_t[:], w_s[:])
                    w_ts.append(w_t)
                for c0 in range(0, T, M_CHUNK):
                    mc = min(M_CHUNK, T - c0)
                    m0 = row0 + c0 * P
                    x_s = xs.tile([P, KK * CH], mybir.dt.float16, tag="xst")
                    nc.sync.dma_start(
                        out=x_s[:], in_=xsw[chunk * P:(chunk + 1) * P, :])
                    x_t = xp.tile([P, KK * CH], mybir.dt.float32r, tag="x")
                    nc.gpsimd.tensor_copy(x_t[:], x_s[:])
                    chunk += 1
                    for s in range(mc):
                        o_t = op.tile([P, NC_N], mybir.dt.float32, tag="o")
                        pss = [pp.tile([P, MM_N], mybir.dt.float32,
                                       tag="ps", name="ps")
                               for _ in range(NH)]
                        for kk in range(KK):
                            for h in range(NH):
                                nc.tensor.matmul(
                                    pss[h][:],
                                    x_t[:, kk * CH + s * P:
                                           kk * CH + s * P + P],
                                    w_ts[kk][:, h * MM_N:(h + 1) * MM_N],
                                    start=(kk == 0), stop=(kk == KK - 1))
                        for h in range(NH):
                            nc.vector.tensor_copy(
                                o_t[:, h * MM_N:(h + 1) * MM_N], pss[h][:])
                        nc.gpsimd.dma_start(
                            out=out[m0 + s * P:m0 + (s + 1) * P, :],
                            in_=o_t[:])
                row0 += T * P
    nc.compile()
    return nc


_NC_CACHE = {}


def get_nc(t_list, K, NC_N, **kw):
    key = (tuple(t_list), K, NC_N, tuple(sorted(kw.items())))
    if key not in _NC_CACHE:
        _NC_CACHE[key] = build_nc(t_list, K, NC_N, **kw)
    return _NC_CACHE[key]


def pack_x(x_padded, order, t_e, K):
    """Swizzled x: row (chunk*P + p) = partition p's contiguous line."""
    KK = K // P
    parts = []
    for e in order:
        T = t_e[e]
        Rp = ((T + M_CHUNK - 1) // M_CHUNK) * CH
        xe = np.zeros((Rp, K), dtype=np.float32)
        xe[:T * P] = x_padded[e, :T * P, :]
        nch = Rp // CH
        a = xe.reshape(nch, CH, KK, P).transpose(0, 3, 2, 1)
        parts.append(np.ascontiguousarray(a).reshape(nch * P, KK * CH))
    return to_fp16(np.concatenate(parts, axis=0))


def pack_w(stacked_weights, order, c, NC_N, K):
    """Per-core swizzled weights: row ((seg*KK+kk)*P + p) = partition
    p's contiguous line for that (segment, k-tile)."""
    KK = K // P
    parts = []
    for e in order:
        blk = stacked_weights[e, c * NC_N:(c + 1) * NC_N, :]
        a = blk.reshape(NC_N, KK, P).transpose(1, 2, 0)
        parts.append(np.ascontiguousarray(a).reshape(KK * P, NC_N))
    return to_fp16(np.concatenate(parts, axis=0))


def kernel(x_padded, stacked_weights, m_sizes):
    global LAST_RESULT
    x_padded = np.ascontiguousarray(np.asarray(x_padded, dtype=np.float32))
    stacked_weights = np.ascontiguousarray(
        np.asarray(stacked_weights, dtype=np.float32))
    E, MAX_M, K = x_padded.shape
    N = stacked_weights.shape[1]
    NC_N = N // N_CORES
    m = np.asarray(m_sizes).astype(np.int64)
    t_e = [min(int(math.ceil(mm / P)), (MAX_M + P - 1) // P) if mm > 0 else 0
           for mm in m]

    out_full = np.zeros((E, MAX_M, N), dtype=np.float32)
    order = [e for e in range(E) if t_e[e] > 0]
    if not order:
        return out_full
    # descending size: big early segments build weight-prefetch credit
    # that carries the small trailing segments without PE stalls
    order.sort(key=lambda e: -t_e[e])
    t_list = [t_e[e] for e in order]

    _install_profile_shim()
    nc = get_nc(t_list, K, NC_N)

    xsw = pack_x(x_padded, order, t_e, K)
    in_maps = [{"xsw": xsw,
                "wsw": pack_w(stacked_weights, order, c, NC_N, K)}
               for c in range(N_CORES)]

    res = run_bass_kernel_spmd(nc, in_maps, list(range(N_CORES)))
    LAST_RESULT = res

    for c in range(N_CORES):
        o = res.results[c]["out"]
        row = 0
        for i, e in enumerate(order):
            rows = int(min(m[e], MAX_M))
            out_full[e, :rows, c * NC_N:(c + 1) * NC_N] = o[row:row + rows]
            row += t_list[i] * P
    return out_full

